# revision 43
# baseline (speedup 1.0000x reference)
"""Trainium2 Bass kernel: GroupNorm + spatial self-attention block.

Per batch item (B=32, C=512, H=W=32, S=H*W=1024):
    h  = GroupNorm(x; 32 groups)
    q/k/v = proj(h); atten = softmax(q k^T / sqrt(C)); o = atten v
    y  = proj_o(o) + x

Sharding: data-parallel over batch across 8 NeuronCores (4 items each).

All five large matmul groups run in fp8-e4m3 with MatmulPerfMode.DoubleRow
(two contraction planes of 128 per instruction, K=256), ~1.4-2x the bf16
stream rate.  Layouts keep every operand in its natural [part, free] form:
  - tn/qT/kT/oT: [c(part), chunk, s]; v: [s(part), chunk, c];
    expT: [t(part), chunk, s].  The chunk dim makes [:, 2k:2k+2, cols] the
    3D [128, 2, N] access pattern DoubleRow requires.
  - Weights are host-packed [p, co*2+k2, plane, m] so each stationary is a
    direct [128, 2, 128] slice.
  - exp is computed as exp(SCALE*score - 2.0): the TRN e4m3 max is 240 and
    conversion overflows to Inf, so the shift keeps exp <= ~170 for this
    input scale; the shift cancels exactly because the softmax denominator
    is summed from the *same* shifted/quantized expT (ones-matmul on PE).
  - 1/d via exp(-ln d) on ACT; the normalization multiplies the av-matmul
    evacuation; v/o biases commute past the output projection (host folds
    Wo@bv + bo).
PSUM accumulators are [128, 1024] (two banks) so every evacuation handles
1024 columns in one instruction.  Engine budget per item: PE all matmuls;
ACT: GN apply, q bias, exp, ln/exp; DVE: k bias, v copy, oT*recipd,
yout(+bias+residual), GN sums; GPSIMD: GN sum-of-squares (items 1..3).
GroupNorm of item b+1 is emitted inside item b's attention so its
DVE/GPSIMD/ACT work hides under the PE stream.
"""

import numpy as np

B, C, H, W = 32, 512, 32, 32
S = H * W  # 1024
N_CORES = 8
BPC = B // N_CORES  # batches per core
G = 32  # groups
CPG = C // G  # channels per group (16)
EPS = 1e-6
SCALE = 0.044194173824159216  # 1/sqrt(512)
CSH = 2.0  # exp shift: exp(SCALE*s - CSH); cancels in softmax

_CACHE = {}


def _split_multiwaits(nc, mybir):
    """This toolchain's walrus crashes (setupSyncWait) on instructions
    carrying more than one sem-wait.  Hoist extras into standalone
    EventSemaphore waits placed just before, preserving per-engine order."""
    for fn in nc.m.functions:
        for bb in fn.blocks:
            new_insts = []
            changed = False
            for inst in bb.instructions:
                si = getattr(inst, "sync_info", None)
                waits = list(si.on_wait) if si is not None else []
                if len(waits) > 1:
                    changed = True
                    for j, w in enumerate(waits[:-1]):
                        ev = mybir.InstEventSemaphore(
                            name=f"{inst.name}_hoistw{j}", ins=[], outs=[]
                        )
                        ev.engine = inst.engine
                        ev.sync_info = mybir.SyncInfo(on_wait=[w], on_update=[])
                        new_insts.append(ev)
                    inst.sync_info = mybir.SyncInfo(
                        on_wait=[waits[-1]], on_update=list(si.on_update)
                    )
                new_insts.append(inst)
            if changed:
                bb.instructions = new_insts


def _build_nc():
    import concourse.bass as bass
    import concourse.tile as tile
    from concourse import mybir
    from contextlib import ExitStack

    f32 = mybir.dt.float32
    fp8 = mybir.dt.float8e4
    DR = mybir.MatmulPerfMode.DoubleRow
    AF = mybir.ActivationFunctionType
    ALU = mybir.AluOpType
    AX = mybir.AxisListType

    nc = bass.Bass()
    bf16 = mybir.dt.bfloat16
    x_d = nc.dram_tensor("xb", [BPC, C, S], bf16, kind="ExternalInput")
    y_d = nc.dram_tensor("y", [BPC, C, S], f32, kind="ExternalOutput")
    wq_d = nc.dram_tensor("wq8", [128, 8, 2, 128], fp8, kind="ExternalInput")
    wk_d = nc.dram_tensor("wk8", [128, 8, 2, 128], fp8, kind="ExternalInput")
    wv_d = nc.dram_tensor("wv8", [128, 2, 2, 512], fp8, kind="ExternalInput")
    wo_d = nc.dram_tensor("wo8", [128, 8, 2, 128], fp8, kind="ExternalInput")
    # all small per-channel constants packed [p, 4*k+ci]:
    # k: 0=bq 1=bk 2=bo2 3=gn_w 4=gn_b
    cc_d = nc.dram_tensor("cc", [128, 20], f32, kind="ExternalInput")
    indf_d = nc.dram_tensor("indf", [128, 4 * G], f32, kind="ExternalInput")
    indb_d = nc.dram_tensor("indb", [G, 4 * 128], f32, kind="ExternalInput")
    ones_d = nc.dram_tensor("ones8", [128, 2, 128], fp8, kind="ExternalInput")

    with tile.TileContext(nc) as tc, ExitStack() as ctx:
        cp = ctx.enter_context(tc.tile_pool(name="consts", bufs=1))
        xp = ctx.enter_context(tc.tile_pool(name="x", bufs=2))
        tn_p = ctx.enter_context(tc.tile_pool(name="tn", bufs=2))
        qk_p = ctx.enter_context(tc.tile_pool(name="qk", bufs=2))
        v_p = ctx.enter_context(tc.tile_pool(name="v", bufs=2))
        e_p = ctx.enter_context(tc.tile_pool(name="expT", bufs=2))
        o_p = ctx.enter_context(tc.tile_pool(name="oT", bufs=2))
        y_p = ctx.enter_context(tc.tile_pool(name="yout", bufs=2))
        sp = ctx.enter_context(tc.tile_pool(name="small", bufs=2))
        ps_mm = ctx.enter_context(tc.tile_pool(name="ps_mm", bufs=3, space="PSUM"))
        ps_gn = ctx.enter_context(tc.tile_pool(name="ps_gn", bufs=2, space="PSUM"))

        # ---- first batch's x loads go ahead of the (bigger) weight DMAs,
        # one chunk per engine queue so they transfer in parallel ----
        x_tiles = {}
        dma_engines = [nc.sync, nc.gpsimd, nc.scalar, nc.sync]
        x_tiles[0] = xp.tile([128, 4, 1024], bf16, tag="x", name="x_sb")
        for ci in range(4):
            dma_engines[ci].dma_start(
                x_tiles[0][:, ci, :],
                x_d[0, ci * 128 : (ci + 1) * 128, :],
            )

        # ---- constants: small ones first (GN needs them ~10us before the
        # projections need the weight matrices) ----
        cc_sb = cp.tile([128, 20], f32, tag="cc")
        nc.sync.dma_start(cc_sb[:], cc_d[:])
        bq_sb = cc_sb[:, 0:4]
        bk_sb = cc_sb[:, 4:8]
        bo_sb = cc_sb[:, 8:12]
        gw_sb = cc_sb[:, 12:16]
        gb_sb = cc_sb[:, 16:20]
        indf_sb = cp.tile([128, 4 * G], f32, tag="indf")
        indb_sb = cp.tile([G, 4 * 128], f32, tag="indb")
        nc.sync.dma_start(indf_sb[:], indf_d[:])
        nc.sync.dma_start(indb_sb[:], indb_d[:])
        ones_sb = cp.tile([128, 2, 128], fp8, tag="ones")
        nc.sync.dma_start(ones_sb[:], ones_d[:])
        eps_sb = cp.tile([G, 1], f32, tag="eps")
        nc.gpsimd.memset(eps_sb[:], EPS)
        csh_sb = cp.tile([128, 1], f32, tag="csh")
        nc.gpsimd.memset(csh_sb[:], -CSH)
        # weights: [p, co*2+k2, plane, m] (q/k/o) or [p, k2, plane, co] (v)
        wq_sb = cp.tile([128, 8, 2, 128], fp8, tag="wq")
        wk_sb = cp.tile([128, 8, 2, 128], fp8, tag="wk")
        wo_sb = cp.tile([128, 8, 2, 128], fp8, tag="wo")
        wv_sb = cp.tile([128, 2, 2, 512], fp8, tag="wv")
        for w_sb, w_d in ((wq_sb, wq_d), (wk_sb, wk_d), (wv_sb, wv_d), (wo_sb, wo_d)):
            nc.sync.dma_start(w_sb[:], w_d[:])

        tn_tiles = {}

        def gn_stats(b, stats_in, first):
            """sums + sum-of-squares.  Item 0 splits across DVE/ACT for
            latency; later items keep everything off ACT (its queue-tail
            position of the GN apply gates the next item's projections).
            GPSIMD can't help: this toolchain's walrus rejects tensor ops on
            the Pool engine."""
            x_sb = x_tiles[b]
            scratch = sp.tile([128, 1024], f32, tag="scratch")
            # scratch is DVE-written, scratch2 ACT-written (sharing one tile
            # would serialize the engines on a false WAW dependency)
            scratch2 = sp.tile([128, 1024], f32, tag="scratch2")
            for ci in range(4):
                xv = x_sb[:, ci, :]
                sum_sl = stats_in[:, 2 * ci : 2 * ci + 1]
                sq_sl = stats_in[:, 2 * ci + 1 : 2 * ci + 2]
                if first and ci % 2 == 1:
                    # item 0: alternate chunks across engines so the two
                    # serial stats chains run in parallel with DMA arrival
                    nc.scalar.activation(scratch2[:], xv, AF.Identity,
                                         accum_out=sum_sl)
                    nc.vector.scalar_tensor_tensor(
                        scratch[:], xv, 0.0, xv,
                        op0=ALU.bypass, op1=ALU.mult, accum_out=sq_sl,
                    )
                else:
                    nc.vector.reduce_sum(sum_sl, xv, axis=AX.X)
                    if first or ci < 2:
                        nc.scalar.activation(scratch2[:], xv, AF.Square,
                                             accum_out=sq_sl)
                    else:
                        nc.vector.scalar_tensor_tensor(
                            scratch[:], xv, 0.0, xv,
                            op0=ALU.bypass, op1=ALU.mult, accum_out=sq_sl,
                        )

        def gn_combine_apply(b, stats_in, first=False):
            """group stats -> per-channel scale/bias (tiny PE/DVE/ACT work),
            then the ACT apply pass producing tn in fp8."""
            x_sb = x_tiles[b]
            gs_ps = ps_gn.tile([G, 2], f32, tag="gn", name="gs_ps")
            for ci in range(4):
                nc.tensor.matmul(
                    gs_ps[:],
                    indf_sb[:, ci * G : (ci + 1) * G],
                    stats_in[:, 2 * ci : 2 * ci + 2],
                    start=(ci == 0), stop=(ci == 3),
                )
            mu_ex = sp.tile([G, 2], f32, tag="mu_ex")
            nc.vector.tensor_scalar_mul(mu_ex[:], gs_ps[:], 1.0 / (CPG * S))
            musq = sp.tile([G, 1], f32, tag="musq")
            var = sp.tile([G, 1], f32, tag="var")
            std = sp.tile([G, 1], f32, tag="std")
            nc.vector.tensor_mul(musq[:], mu_ex[:, 0:1], mu_ex[:, 0:1])
            nc.vector.tensor_sub(var[:], mu_ex[:, 1:2], musq[:])
            nc.scalar.activation(std[:], var[:], AF.Sqrt, bias=eps_sb[:, 0:1])
            stats2 = sp.tile([G, 2], f32, tag="stats2")
            nc.vector.reciprocal(stats2[:, 1:2], std[:])
            nc.vector.tensor_copy(stats2[:, 0:1], mu_ex[:, 0:1])

            # broadcast g->c in one [128, 8] psum tile, then compute all four
            # chunks' scale/bias with three strided DVE ops (the per-chunk
            # DVE<->PE ping-pong was ~4us of serial latency)
            scl = sp.tile([128, 4], f32, tag="scl")
            bia = sp.tile([128, 4], f32, tag="bia")
            tmp4 = sp.tile([128, 4], f32, tag="tmp4")
            bc_ps = ps_gn.tile([128, 8], f32, tag="gn", name="bc_ps")
            for ci in range(4):
                nc.tensor.matmul(
                    bc_ps[:, 2 * ci : 2 * ci + 2],
                    indb_sb[:, ci * 128 : (ci + 1) * 128],
                    stats2[:],
                    start=True, stop=True,
                )
            nc.vector.tensor_mul(scl[:], bc_ps[:, 1:8:2], gw_sb[:])
            nc.vector.tensor_mul(tmp4[:], bc_ps[:, 0:8:2], scl[:])
            nc.vector.tensor_sub(bia[:], gb_sb[:], tmp4[:])
            tn_sb = tn_tiles[b] = tn_p.tile([128, 4, 1024], fp8, tag="tn",
                                            name="tn_sb")
            for ci in range(4):
                if first and ci % 2 == 1:
                    # item 0: split the apply across both engines (nothing
                    # else is running yet and it gates the first projection)
                    nc.vector.tensor_scalar(
                        tn_sb[:, ci, :], x_sb[:, ci, :],
                        scl[:, ci : ci + 1], bia[:, ci : ci + 1],
                        op0=ALU.mult, op1=ALU.add,
                    )
                else:
                    nc.scalar.activation(
                        tn_sb[:, ci, :],
                        x_sb[:, ci, :],
                        AF.Identity,
                        bias=bia[:, ci : ci + 1], scale=scl[:, ci : ci + 1],
                    )

        mid_state = {}

        def attn_phase_a(b):
            tn_sb = tn_tiles.pop(b)

            # prefetch x of the next item; its GN stats are emitted below,
            # after the k/v evacuations, so the DVE queue never head-of-line
            # blocks on this DMA
            if b + 1 < BPC and b + 1 not in x_tiles:
                x_nb = x_tiles[b + 1] = xp.tile([128, 4, 1024], bf16, tag="x",
                                                name="x_sb")
                for ci in range(4):
                    dma_engines[ci].dma_start(
                        x_nb[:, ci, :],
                        x_d[b + 1, ci * 128 : (ci + 1) * 128, :],
                    )

            # ---- q/k (channel-partitioned): out[co, s] over K=c_in ----
            qT = qk_p.tile([128, 4, 1024], fp8, tag="qT")
            kT = qk_p.tile([128, 4, 1024], fp8, tag="kT")
            for w_sb, b_sb, dst, eng in (
                (wq_sb, bq_sb, qT, "act"), (wk_sb, bk_sb, kT, "dve"),
            ):
                for co in range(4):
                    mm = ps_mm.tile([128, 1024], f32, tag="mm", name="mmqk")
                    for k2 in range(2):
                        for ch in range(2):
                            nc.tensor.matmul(
                                mm[:, ch * 512 : (ch + 1) * 512],
                                w_sb[:, co * 2 + k2],
                                tn_sb[:, 2 * k2 : 2 * k2 + 2,
                                      ch * 512 : ch * 512 + 512],
                                start=(k2 == 0), stop=(k2 == 1),
                                perf_mode=DR,
                            )
                    if eng == "act":
                        nc.scalar.activation(dst[:, co, :], mm[:], AF.Identity,
                                             bias=b_sb[:, co : co + 1])
                    else:
                        nc.vector.tensor_scalar_add(
                            dst[:, co, :], mm[:], b_sb[:, co : co + 1]
                        )

            # ---- v (position-partitioned): out[si, c_out] over K=c_in ----
            v_sb = v_p.tile([128, 8, 512], fp8, tag="v")
            for sp_ in range(4):
                mm = ps_mm.tile([128, 1024], f32, tag="mm", name="mmv")
                for half in range(2):
                    si = 2 * sp_ + half
                    for k2 in range(2):
                        nc.tensor.matmul(
                            mm[:, half * 512 : (half + 1) * 512],
                            tn_sb[:, 2 * k2 : 2 * k2 + 2,
                                  si * 128 : si * 128 + 128],
                            wv_sb[:, k2],
                            start=(k2 == 0), stop=(k2 == 1),
                            perf_mode=DR,
                        )
                nc.vector.tensor_copy(v_sb[:, 2 * sp_ : 2 * sp_ + 2, :], mm[:])

            # next item's GN stats: emitted here (after the k/v evacuations)
            # — measured faster than after-scores or priority-hinted variants
            stats_nb = None
            if b + 1 < BPC:
                stats_nb = sp.tile([128, 8], f32, tag="stats_in")
                gn_stats(b + 1, stats_nb, first=False)

            # ---- scoresT + exp ----
            expT = e_p.tile([128, 8, 1024], fp8, tag="expT")
            for ti in range(8):
                mm = ps_mm.tile([128, 1024], f32, tag="mm", name="mms")
                for k2 in range(2):
                    for ch in range(2):
                        nc.tensor.matmul(
                            mm[:, ch * 512 : (ch + 1) * 512],
                            kT[:, 2 * k2 : 2 * k2 + 2,
                               ti * 128 : ti * 128 + 128],
                            qT[:, 2 * k2 : 2 * k2 + 2,
                               ch * 512 : ch * 512 + 512],
                            start=(k2 == 0), stop=(k2 == 1),
                            perf_mode=DR,
                        )
                nc.scalar.activation(expT[:, ti, :], mm[:], AF.Exp,
                                     bias=csh_sb[:, 0:1], scale=SCALE)

            # ---- softmax denominator: ones-matmul column sums (result
            # replicated over partitions) ----
            d_ps = ps_mm.tile([128, 1024], f32, tag="mm", name="dps")
            for t2 in range(4):
                for ch in range(2):
                    nc.tensor.matmul(
                        d_ps[:, ch * 512 : (ch + 1) * 512],
                        ones_sb[:],
                        expT[:, 2 * t2 : 2 * t2 + 2, ch * 512 : ch * 512 + 512],
                        start=(t2 == 0), stop=(t2 == 3),
                        perf_mode=DR,
                    )

            mid_state[b] = (v_sb, expT, d_ps, stats_nb)
            return stats_nb

        def attn_phase_b(b, mid=None):
            x_sb = x_tiles.pop(b)
            v_sb, expT, d_ps, _ = mid_state.pop(b)

            # 1/d as exp(-ln d) on ACT (DVE's iterative reciprocal is ~8
            # cyc/element; far too slow for 1024 values on the critical path)
            lnd = sp.tile([128, 1024], f32, tag="lnd")
            recipd = sp.tile([128, 1024], f32, tag="recipd")
            nc.scalar.activation(lnd[:], d_ps[:], AF.Ln)
            nc.scalar.activation(recipd[:], lnd[:], AF.Exp, scale=-1.0)

            # ---- o = atten @ v (unnormalized), channel-partitioned ----
            oT = o_p.tile([128, 4, 1024], fp8, tag="oT")
            for co in range(4):
                mm = ps_mm.tile([128, 1024], f32, tag="mm", name="mma")
                for t2 in range(4):
                    for ch in range(2):
                        nc.tensor.matmul(
                            mm[:, ch * 512 : (ch + 1) * 512],
                            v_sb[:, 2 * t2 : 2 * t2 + 2,
                                 co * 128 : co * 128 + 128],
                            expT[:, 2 * t2 : 2 * t2 + 2,
                                 ch * 512 : ch * 512 + 512],
                            start=(t2 == 0), stop=(t2 == 3),
                            perf_mode=DR,
                        )
                nc.vector.tensor_mul(oT[:, co, :], mm[:], recipd[:])
                if co == 1 and mid is not None:
                    mid()  # next item's GN combine+apply: PE/DVE/ACT all
                    # have their stats inputs ready by now, and the apply
                    # still lands before the next item's first projection

            # ---- output projection + bias + residual ----
            # last item: evacuate in halves on alternating engines so the
            # final DMA starts as early as possible (epilogue is exposed)
            last = b == BPC - 1
            yout = y_p.tile([128, 4, 1024], f32, tag="yout")
            for co in range(4):
                mm = ps_mm.tile([128, 1024], f32, tag="mm", name="mmo")
                for k2 in range(2):
                    for ch in range(2):
                        nc.tensor.matmul(
                            mm[:, ch * 512 : (ch + 1) * 512],
                            wo_sb[:, co * 2 + k2],
                            oT[:, 2 * k2 : 2 * k2 + 2,
                               ch * 512 : ch * 512 + 512],
                            start=(k2 == 0), stop=(k2 == 1),
                            perf_mode=DR,
                        )
                if last:
                    for ch in range(2):
                        sl = slice(ch * 512, (ch + 1) * 512)
                        nc.vector.scalar_tensor_tensor(
                            yout[:, co, sl], mm[:, sl],
                            bo_sb[:, co : co + 1], x_sb[:, co, sl],
                            op0=ALU.add, op1=ALU.add,
                        )
                        (nc.sync if ch == 0 else nc.gpsimd).dma_start(
                            y_d[b, co * 128 : (co + 1) * 128, sl],
                            yout[:, co, sl],
                        )
                else:
                    nc.vector.scalar_tensor_tensor(
                        yout[:, co, :], mm[:], bo_sb[:, co : co + 1],
                        x_sb[:, co, :],
                        op0=ALU.add, op1=ALU.add,
                    )
                    (nc.sync if co % 2 == 0 else nc.gpsimd).dma_start(
                        y_d[b, co * 128 : (co + 1) * 128, :],
                        yout[:, co, :],
                    )

        def psum_slot_pad(n):
            """Dummy (never-written) psum allocations that advance the pool's
            round-robin so the NEXT item's first q-matmuls land on slots whose
            previous tiles were evacuated EARLY in this item (q/k, ACT), not
            on ones still waiting for the late yout reads on DVE.  32 allocs
            per item keeps the 3-slot rotation phase-aligned."""
            for _ in range(n):
                ps_mm.tile([128, 1024], f32, tag="mm", name="pad")

        # software pipeline: item b+1's GN is emitted inside item b's
        # attention so its DVE/GPSIMD/ACT work hides under the PE stream
        stats0 = sp.tile([128, 8], f32, tag="stats_in")
        gn_stats(0, stats0, first=True)
        with tc.high_priority(offset=40):
            gn_combine_apply(0, stats0, first=True)
        for b in range(BPC):
            stats_nb = attn_phase_a(b)
            mid = None
            if b + 1 < BPC:
                def mid(bb=b + 1, st=stats_nb):
                    with tc.high_priority(offset=40):
                        gn_combine_apply(bb, st)
            attn_phase_b(b, mid=mid)
            psum_slot_pad(3)

    _split_multiwaits(nc, mybir)
    return nc


def _host_consts(gn_w, gn_b, Wq, bq, Wk, bk, Wv, bv, Wo, bo):
    import ml_dtypes
    f = np.float32
    e4 = ml_dtypes.float8_e4m3

    def q8(a):  # TRN e4m3 overflows to Inf above 240; clip first
        return np.clip(np.asarray(a, f), -240.0, 240.0).astype(e4)

    bo2 = (Wo.astype(np.float64) @ bv.astype(np.float64) + bo).astype(f)
    # indf: [p, ci*G+g]; indb: [g, ci*128+c] (chunk-local group indicators)
    indf = np.zeros((128, 4 * G), f)
    indb = np.zeros((G, 4 * 128), f)
    for ci in range(4):
        for c in range(128):
            g = 8 * ci + c // CPG
            indf[c, ci * G + g] = 1.0
            indb[g, ci * 128 + c] = 1.0
    cc = np.stack(
        [bq.astype(f), bk.astype(f), bo2, gn_w.astype(f), gn_b.astype(f)]
    ).reshape(5, 4, 128)  # [k, ci, p]
    cc = np.ascontiguousarray(cc.transpose(2, 0, 1).reshape(128, 20))

    def pack_stat(WT):
        # [p, co*2+k2, pl, m] <- W.T[k2*256 + pl*128 + p, co*128 + m]
        w = np.asarray(WT, f).reshape(2, 2, 128, 4, 128)  # [k2, pl, p, co, m]
        w = w.transpose(2, 3, 0, 1, 4).reshape(128, 8, 2, 128)
        return np.ascontiguousarray(q8(w))

    def pack_mov(WT):
        # [p, k2, pl, co] <- W.T[k2*256 + pl*128 + p, co]
        w = np.asarray(WT, f).reshape(2, 2, 128, 512)  # [k2, pl, p, co]
        w = w.transpose(2, 0, 1, 3).reshape(128, 2, 2, 512)
        return np.ascontiguousarray(q8(w))

    return {
        "wq8": pack_stat(Wq.T),
        "wk8": pack_stat(Wk.T),
        "wo8": pack_stat(Wo.T),
        "wv8": pack_mov(Wv.T),
        "cc": cc,
        "indf": indf,
        "indb": indb,
        "ones8": np.ones((128, 2, 128), e4),
    }


def kernel(x, gn_w, gn_b, Wq, bq, Wk, bk, Wv, bv, Wo, bo, _trace=False):
    from concourse.bass_utils import run_bass_kernel_spmd

    if "nc" not in _CACHE:
        _CACHE["nc"] = _build_nc()
    nc = _CACHE["nc"]

    x = np.asarray(x, np.float32)
    consts = _host_consts(
        np.asarray(gn_w), np.asarray(gn_b),
        np.asarray(Wq), np.asarray(bq),
        np.asarray(Wk), np.asarray(bk),
        np.asarray(Wv), np.asarray(bv),
        np.asarray(Wo), np.asarray(bo),
    )
    import ml_dtypes
    xr = np.ascontiguousarray(x.reshape(B, C, S)).astype(ml_dtypes.bfloat16)
    in_maps = [
        {"xb": np.ascontiguousarray(xr[c * BPC : (c + 1) * BPC]), **consts}
        for c in range(N_CORES)
    ]
    res = run_bass_kernel_spmd(nc, in_maps, list(range(N_CORES)), trace=_trace)
    _CACHE["last_result"] = res
    y = np.concatenate([res.results[c]["y"] for c in range(N_CORES)], axis=0)
    return y.reshape(B, C, H, W)
